# revision 2
# baseline (speedup 1.0000x reference)
"""Trainium2 Bass kernel for DGCNN (gnn_message_passing) — v2.

Data-parallel over graphs: 32 graphs/core x 8 cores, no collectives.

Key differences vs v1 (all aimed at the DVE bottleneck):
  - Host sorts each graph's edges into 4 (d_half, s_half) buckets and pads
    each bucket to a multiple of 128 edges -> one-hots are 128-wide (not
    256) and the C matmul streams 129 cols (128 d + ones col for deg_out).
  - One-hots built pair-interleaved ([128, pairs, 128, 2] with the pair as
    the innermost packed axis) so the DVE is_equal runs in the 2x 16-bit
    mode; matmuls consume stride-2 slices.
  - Whole GNN in bf16 (embedding table, C counts, h, sort) -> 2x DVE on
    eligible ops, 1-cycle/row PE matmuls instead of 4 (fp32).
  - deg_out free via the ones column; deg_in via lhsT=C-chunk rhs=ones
    matmuls into column slices of one [128, 64] psum tile.
  - norm_src folded into C once; norm_dst applied per layer as the
    per-partition ACT scale during the psum->SBUF copy.
  - Bitonic sort in bf16 with direction-partitioned all-forward slices
    (2x for d>=2 substages).

kernel(**inputs) takes FULL unsharded inputs, shards on host (index
marshalling only), runs one compiled program SPMD on cores 0-7, concats
per-core outputs.
"""

import os

import numpy as np

import concourse.bass as bass
import concourse.bacc as bacc
import concourse.mybir as mybir
import concourse.tile as tile
from concourse.masks import make_identity

F32 = mybir.dt.float32
BF16 = mybir.dt.bfloat16
F16 = mybir.dt.float16
I16 = mybir.dt.int16
U16 = mybir.dt.uint16
ALU = mybir.AluOpType
ACTF = mybir.ActivationFunctionType
AX = mybir.AxisListType

B, NPER, DEG, H, L, K = 256, 256, 16, 128, 3, 30
NCORES = 8
GPC = B // NCORES            # 32 graphs per core
NPC = GPC * NPER             # 8192 nodes per core
EPC = NPC * DEG              # 131072 real edges per core
NEG = -2.0                   # below any tanh output
PAD = 60000.0                # sorts to the end (fits fp16)
NROWS = GPC * K              # 960 pooled rows
RBLK = 8                     # ceil(960/128) row blocks
DUMMY = 200                  # local id outside [0,128) -> zero one-hot


def build_nc(NB):
    """NB = blocks (of 128 edge slots) per (graph, bucket); 4 buckets."""
    NBLK_G = 4 * NB          # edge blocks per graph
    PAIRS = NBLK_G // 2      # is_equal pair count per graph
    nc = bacc.Bacc(None)

    # ---- inputs (host-marshalled shards; see shard_inputs) ----
    z_idx = nc.dram_tensor("z_idx", [128, NPC // 16], I16, kind="ExternalInput")
    s_arr = nc.dram_tensor("s_arr", [128, GPC * PAIRS * 2], BF16,
                           kind="ExternalInput")
    d_arr = nc.dram_tensor("d_arr", [128, GPC * PAIRS * 2], BF16,
                           kind="ExternalInput")
    z_table = nc.dram_tensor("z_table", [1000, H], F32, kind="ExternalInput")
    biases = nc.dram_tensor("biases", [L, H], F32, kind="ExternalInput")
    w1 = nc.dram_tensor("w1", [16, 384], F32, kind="ExternalInput")
    b1 = nc.dram_tensor("b1", [16, 1], F32, kind="ExternalInput")
    w2m = nc.dram_tensor("w2m", [32, 80], F32, kind="ExternalInput")
    b2 = nc.dram_tensor("b2", [32, 1], F32, kind="ExternalInput")
    lw1m = nc.dram_tensor("lw1m", [128, 352], F32, kind="ExternalInput")
    lb1 = nc.dram_tensor("lb1", [128, 1], F32, kind="ExternalInput")
    lw2 = nc.dram_tensor("lw2", [128, 1], F32, kind="ExternalInput")
    lb2 = nc.dram_tensor("lb2", [1, 1], F32, kind="ExternalInput")
    out_d = nc.dram_tensor("out", [GPC, 1], F32, kind="ExternalOutput")

    # ---- DRAM scratch ----
    dbg = os.environ.get("KERNEL_DEBUG", "0") == "1"
    xk = "ExternalOutput" if dbg else "Internal"
    x_d = [nc.dram_tensor(f"x{l}_scratch", [NPC, H], F32, kind=xk)
           for l in range(L)]
    ids_d = nc.dram_tensor("ids_scratch", [NROWS], I16, kind=xk)
    if dbg:
        dga_d = nc.dram_tensor("dga_dbg", [128, 2 * GPC], F32,
                               kind="ExternalOutput")
        din_d = nc.dram_tensor("din_dbg", [128, 2 * GPC], F32,
                               kind="ExternalOutput")
        xs_d = nc.dram_tensor("xs_dbg", [128, RBLK, 512], F16,
                              kind="ExternalOutput")
    idsw_d = nc.dram_tensor("idsw_scratch", [128, NROWS // 16], I16,
                            kind="Internal")

    with tile.TileContext(nc) as tc:
        with (
            tc.tile_pool(name="big", bufs=1) as big,
            tc.tile_pool(name="work", bufs=2) as work,
            tc.tile_pool(name="small", bufs=1) as small,
            tc.tile_pool(name="psC", bufs=4, space="PSUM") as psC,
            tc.tile_pool(name="psL", bufs=2, space="PSUM") as psL,
            tc.tile_pool(name="psum1", bufs=1, space="PSUM") as psum1,
        ):
            # ---------- constants ----------
            ident = small.tile([128, 128], F32, tag="ident")
            make_identity(nc, ident[:])
            identb = small.tile([128, 128], F16, tag="identb")
            nc.vector.tensor_copy(identb[:], ident[:])
            io16 = small.tile([128, 128, 2], I16, tag="io16")
            nc.gpsimd.iota(io16[:], pattern=[[1, 128], [0, 2]], base=0,
                           channel_multiplier=0)
            iota2 = small.tile([128, 128, 2], BF16, tag="iota2")
            nc.vector.tensor_copy(iota2[:], io16[:])
            ones_b = small.tile([128, 1], F32, tag="ones")
            nc.vector.memset(ones_b[:], 1.0)
            bias_rep = small.tile([128, L, H], F32, tag="brep")
            for l in range(L):
                nc.sync.dma_start(out=bias_rep[:, l, :],
                                  in_=biases[l:l + 1, :].broadcast_to([128, H]))

            # ---------- embedding gather: h0 = z_table[z] ----------
            h_a = big.tile([128, 2 * GPC, H], F32, tag="ha")
            h_b = big.tile([128, 2 * GPC, H], F32, tag="hb")
            zi = small.tile([128, NPC // 16], I16, tag="zi")
            nc.sync.dma_start(out=zi[:], in_=z_idx[:])
            nc.gpsimd.dma_gather(
                out_ap=h_a[:], in_ap=z_table[:], idxs_ap=zi[:],
                num_idxs=NPC, num_idxs_reg=NPC, elem_size=H,
                single_packet=False)

            # ---------- C build ----------
            # ct_sb chunk (gl, dh, sh): C[s_loc, d_loc] for s in half sh,
            # d in half dh; col 128 = deg_out partial (sum over dh half).
            ct_sb = big.tile([128, GPC * 4, 129], F32, tag="ct")
            sa = small.tile([128, GPC * PAIRS, 2], BF16, tag="sa")
            da = small.tile([128, GPC * PAIRS, 2], BF16, tag="da")
            nc.sync.dma_start(
                out=sa[:], in_=s_arr[:].rearrange("p (a b) -> p a b", b=2))
            nc.sync.dma_start(
                out=da[:], in_=d_arr[:].rearrange("p (a b) -> p a b", b=2))

            HP = NB  # pairs per dh-half (= 2*NB blocks = 2 buckets)
            for gl in range(GPC):
                for dh in range(2):
                    soh = work.tile([128, HP, 128, 2], BF16, tag="soh",
                                    name=f"soh{gl}_{dh}")
                    doh = work.tile([128, HP, 129, 2], BF16, tag="doh",
                                    name=f"doh{gl}_{dh}")
                    nc.vector.memset(doh[:, :, 128, :], 1.0)
                    p0 = gl * PAIRS + dh * HP
                    nc.vector.tensor_tensor(
                        out=soh[:],
                        in0=sa[:, p0:p0 + HP, :].unsqueeze(2)
                        .broadcast_to([128, HP, 128, 2]),
                        in1=iota2[:].unsqueeze(1).broadcast_to(
                            [128, HP, 128, 2]),
                        op=ALU.is_equal)
                    nc.vector.tensor_tensor(
                        out=doh[:, :, 0:128, :],
                        in0=da[:, p0:p0 + HP, :].unsqueeze(2)
                        .broadcast_to([128, HP, 128, 2]),
                        in1=iota2[:].unsqueeze(1).broadcast_to(
                            [128, HP, 128, 2]),
                        op=ALU.is_equal)
                    pcs = psC.tile([128, 2, 129], F32, tag="cb",
                                   name=f"cb{gl}_{dh}")
                    for bl in range(2 * NB):
                        sh = bl // NB
                        bp, j = bl // 2, bl % 2
                        nc.tensor.matmul(
                            out=pcs[:, sh, :],
                            lhsT=soh[:, bp, :, j],
                            rhs=doh[:, bp, :, j],
                            start=(bl % NB == 0), stop=(bl % NB == NB - 1))
                    nc.scalar.copy(
                        ct_sb[:, gl * 4 + dh * 2:gl * 4 + dh * 2 + 2, :],
                        pcs[:])

            # ---------- degrees ----------
            ctv = ct_sb[:].rearrange("p (g d s) c -> p g d s c", d=2, s=2)
            dga = small.tile([128, 2 * GPC], F32, tag="dga")
            nc.vector.tensor_tensor(
                out=dga[:].rearrange("p (g s) -> p g s", s=2),
                in0=ctv[:, :, 0, :, 128], in1=ctv[:, :, 1, :, 128],
                op=ALU.add)
            # deg_in[d, (gl,dh)] via lhsT=C chunk, rhs=ones column
            pdi = psL.tile([128, 2 * GPC], F32, tag="mm", name="pdi")
            for gl in range(GPC):
                for dh in range(2):
                    for sh in range(2):
                        nc.tensor.matmul(
                            out=pdi[:, gl * 2 + dh:gl * 2 + dh + 1],
                            lhsT=ct_sb[:, gl * 4 + dh * 2 + sh, 0:128],
                            rhs=ones_b[:],
                            start=(sh == 0), stop=(sh == 1))
            deg_in = small.tile([128, 2 * GPC], F32, tag="degin")
            nc.scalar.copy(deg_in[:], pdi[:])
            nsrc = small.tile([128, 2 * GPC], F32, tag="nsrc")
            ndst = small.tile([128, 2 * GPC], F32, tag="ndst")
            _rsqrt(nc, small, nsrc, dga, "a")
            _rsqrt(nc, small, ndst, deg_in, "b")
            # fold norm_src into C: chunk (gl, dh, sh) *= nsrc[:, (gl, sh)]
            for gl in range(GPC):
                for sh in range(2):
                    v = ctv[:, gl, :, sh, :]
                    nc.vector.tensor_scalar_mul(
                        out=v, in0=v,
                        scalar1=nsrc[:, gl * 2 + sh:gl * 2 + sh + 1])

            # ---------- layers ----------
            rmax = [small.tile([128, 2 * GPC], F32, tag=f"rmax{l}",
                               name=f"rmax{l}") for l in range(L)]
            hs = [h_a, h_b]
            for l in range(L):
                h, hn = hs[l % 2], hs[1 - l % 2]
                for gl in range(GPC):
                    for dh in range(2):
                        pl = psL.tile([128, H], F32, tag="mm",
                                      name=f"mm{l}_{gl}_{dh}")
                        for sh in range(2):
                            nc.tensor.matmul(
                                out=pl[:],
                                lhsT=ct_sb[:, gl * 4 + dh * 2 + sh, 0:128],
                                rhs=h[:, gl * 2 + sh, :],
                                start=(sh == 0), stop=(sh == 1))
                        r = gl * 2 + dh
                        nc.scalar.activation(hn[:, r, :], pl[:], ACTF.Copy,
                                             scale=ndst[:, r:r + 1])
                nc.vector.tensor_tensor(
                    out=hn[:], in0=hn[:],
                    in1=bias_rep[:, l, :].unsqueeze(1)
                    .broadcast_to([128, 2 * GPC, H]),
                    op=ALU.add)
                nc.scalar.activation(hn[:], hn[:], ACTF.Tanh)
                nc.vector.tensor_reduce(out=rmax[l][:], in_=hn[:],
                                        axis=AX.X, op=ALU.max)
                nc.sync.dma_start(
                    out=x_d[l][:].rearrange("(b p) m -> p b m", p=128),
                    in_=hn[:])

            # ---------- top-30 per graph ----------
            nc.vector.tensor_tensor(out=rmax[0][:], in0=rmax[0][:],
                                    in1=rmax[1][:], op=ALU.max)
            nc.vector.tensor_tensor(out=rmax[0][:], in0=rmax[0][:],
                                    in1=rmax[2][:], op=ALU.max)
            gm = small.tile([GPC, NPER], F32, tag="gm")
            for s in range(2):
                ptr = psum1.tile([GPC, 128], F32, tag="cc", name="tpk")
                nc.tensor.transpose(
                    out=ptr[:],
                    in_=rmax[0][:].rearrange("p (g s) -> p s g", s=2)[:, s],
                    identity=ident[:])
                nc.vector.tensor_copy(gm[:, s * 128:(s + 1) * 128], ptr[:])
            ids = small.tile([GPC, 32], U16, tag="ids")
            vals8 = small.tile([GPC, 8], F32, tag="vals8")
            for r in range(4):
                nc.vector.max(out=vals8[:], in_=gm[:])
                nc.vector.max_index(out=ids[:, r * 8:(r + 1) * 8],
                                    in_max=vals8[:], in_values=gm[:])
                nc.vector.match_replace(out=gm[:], in_to_replace=vals8[:],
                                        in_values=gm[:], imm_value=NEG)
            gid = small.tile([GPC, 32], I16, tag="gid")
            goff = small.tile([GPC, 1], I16, tag="goff")
            nc.gpsimd.iota(goff[:], pattern=[[1, 1]], base=0,
                           channel_multiplier=NPER)
            nc.vector.tensor_tensor(out=gid[:], in0=ids[:],
                                    in1=goff[:].broadcast_to([GPC, 32]),
                                    op=ALU.add)
            nc.sync.dma_start(
                out=ids_d[:].rearrange("(g k) -> g k", g=GPC),
                in_=gid[:, 0:K])
            gw16 = small.tile([16, NROWS // 16], I16, tag="gw16")
            nc.sync.dma_start(
                out=gw16[:], in_=ids_d[:].rearrange("(t p) -> p t", p=16))
            for r in range(8):
                nc.sync.dma_start(out=idsw_d[r * 16:(r + 1) * 16, :],
                                  in_=gw16[:])
            gidx = small.tile([128, NROWS // 16], I16, tag="gidx")
            nc.sync.dma_start(out=gidx[:], in_=idsw_d[:])

            # ---------- gather pooled rows + sort (bf16) ----------
            xs = big.tile([128, RBLK, 512], F16, tag="xs")
            xs2 = big.tile([128, RBLK, 512], F16, tag="xs2")
            nc.vector.memset(xs[:, :, 384:512], PAD)
            for l in range(L):
                gx = small.tile([128, RBLK, H], F32, tag="gx",
                                name=f"gx{l}")
                nc.gpsimd.dma_gather(
                    out_ap=gx[:], in_ap=x_d[l][:], idxs_ap=gidx[:],
                    num_idxs=NROWS, num_idxs_reg=NROWS, elem_size=H)
                nc.vector.tensor_copy(xs[:, :, l * H:(l + 1) * H], gx[:])
            xs_fin = _bitonic_sort(nc, xs, xs2)
            if dbg:
                nc.sync.dma_start(out=dga_d[:], in_=dga[:])
                nc.sync.dma_start(out=din_d[:], in_=deg_in[:])
                nc.sync.dma_start(out=xs_d[:], in_=xs_fin[:])

            # ---------- conv tail ----------
            pooled_t = [small.tile([128, RBLK * 128], F32, tag=f"pt{j}",
                                   name=f"pt{j}") for j in range(3)]
            for j in range(3):
                for bb in range(RBLK):
                    ptr = psum1.tile([128, 128], F16, tag="tp2b")
                    nc.tensor.transpose(
                        out=ptr[:], in_=xs_fin[:, bb, j * 128:(j + 1) * 128],
                        identity=identb[:])
                    nc.scalar.copy(
                        pooled_t[j][:, bb * 128:(bb + 1) * 128], ptr[:])
            w1_sb = small.tile([16, 384], F32, tag="w1")
            nc.sync.dma_start(out=w1_sb[:], in_=w1[:])
            w1t = small.tile([128, 3, 16], F32, tag="w1t")
            for j in range(3):
                ptr = psum1.tile([128, 16], F32, tag="cc", name="w1tp")
                nc.tensor.transpose(out=ptr[:],
                                    in_=w1_sb[:, j * 128:(j + 1) * 128],
                                    identity=ident[:16, :16])
                nc.scalar.copy(w1t[:, j, :], ptr[:])
            b1_sb = small.tile([16, 1], F32, tag="b1")
            nc.sync.dma_start(out=b1_sb[:], in_=b1[:])
            out1 = small.tile([16, RBLK * 128], F32, tag="out1")
            for ch in range(2):
                pc = psum1.tile([16, 512], F32, tag="cc")
                for j in range(3):
                    nc.tensor.matmul(
                        out=pc[:], lhsT=w1t[:, j, :],
                        rhs=pooled_t[j][:, ch * 512:(ch + 1) * 512],
                        start=(j == 0), stop=(j == 2))
                nc.scalar.activation(out1[:, ch * 512:(ch + 1) * 512], pc[:],
                                     ACTF.Relu, bias=b1_sb[:, 0:1])
            pmax = small.tile([16, GPC * 15], F32, tag="pmax")
            o1v = out1[:, 0:GPC * 30].rearrange("p (g k) -> p g k", g=GPC)
            o1v = o1v.rearrange("p g (i two) -> p g i two", two=2)
            nc.vector.tensor_tensor(
                out=pmax[:].rearrange("p (g i) -> p g i", g=GPC),
                in0=o1v[:, :, :, 0], in1=o1v[:, :, :, 1], op=ALU.max)
            w2_sb = small.tile([32, 80], F32, tag="w2")
            nc.sync.dma_start(out=w2_sb[:], in_=w2m[:])
            w2t = small.tile([16, 5, 32], F32, tag="w2t")
            for dt in range(5):
                ptr = psum1.tile([16, 32], F32, tag="cc", name="w2tp")
                nc.tensor.transpose(out=ptr[:],
                                    in_=w2_sb[:, dt * 16:(dt + 1) * 16],
                                    identity=ident[:32, :32])
                nc.scalar.copy(w2t[:, dt, :], ptr[:])
            b2_sb = small.tile([32, 1], F32, tag="b2")
            nc.sync.dma_start(out=b2_sb[:], in_=b2[:])
            out2 = small.tile([32, GPC * 11], F32, tag="out2")
            pv = pmax[:].rearrange("p (g i) -> p g i", g=GPC)
            pc2 = psum1.tile([32, GPC * 11], F32, tag="cc")
            for dt in range(5):
                nc.tensor.matmul(
                    out=pc2[:].rearrange("p (g t) -> p g t", g=GPC),
                    lhsT=w2t[:, dt, :], rhs=pv[:, :, dt:dt + 11],
                    start=(dt == 0), stop=(dt == 4))
            nc.scalar.activation(out2[:], pc2[:], ACTF.Relu,
                                 bias=b2_sb[:, 0:1])
            rhs352 = [small.tile([128, GPC], F32, tag=f"rhs352_{j}",
                                 name=f"rhs352_{j}") for j in range(3)]
            o2v = out2[:].rearrange("p (g t) -> p g t", g=GPC)
            for t in range(11):
                j, r = t // 4, (t % 4) * 32
                nc.vector.tensor_copy(rhs352[j][r:r + 32], o2v[:, :, t])
            lw1_sb = small.tile([128, 352], F32, tag="lw1")
            nc.sync.dma_start(out=lw1_sb[:], in_=lw1m[:])
            lw1t = [small.tile([128, 128], F32, tag=f"lw1t{j}",
                               name=f"lw1t{j}") for j in range(3)]
            for j in range(3):
                w = 128 if j < 2 else 96
                ptr = psum1.tile([128, 128], F32, tag="cc", name=f"lw1tp{j}")
                nc.tensor.transpose(out=ptr[:w, :],
                                    in_=lw1_sb[:, j * 128:j * 128 + w],
                                    identity=ident[:])
                nc.scalar.copy(lw1t[j][:w, :], ptr[:w, :])
            lb1_sb = small.tile([128, 1], F32, tag="lb1")
            nc.sync.dma_start(out=lb1_sb[:], in_=lb1[:])
            h1t = small.tile([128, GPC], F32, tag="h1t")
            pc3 = psum1.tile([128, GPC], F32, tag="cc")
            for j in range(3):
                w = 128 if j < 2 else 96
                nc.tensor.matmul(out=pc3[:], lhsT=lw1t[j][:w, :],
                                 rhs=rhs352[j][:w, :],
                                 start=(j == 0), stop=(j == 2))
            nc.scalar.activation(h1t[:], pc3[:], ACTF.Relu,
                                 bias=lb1_sb[:, 0:1])
            lw2_sb = small.tile([128, 1], F32, tag="lw2")
            nc.sync.dma_start(out=lw2_sb[:], in_=lw2[:])
            lb2_sb = small.tile([GPC, 1], F32, tag="lb2")
            nc.sync.dma_start(out=lb2_sb[:],
                              in_=lb2[:].broadcast_to([GPC, 1]))
            pc4 = psum1.tile([GPC, 1], F32, tag="cc")
            nc.tensor.matmul(out=pc4[:], lhsT=h1t[:], rhs=lw2_sb[:],
                             start=True, stop=True)
            res = small.tile([GPC, 1], F32, tag="res")
            nc.vector.tensor_add(out=res[:], in0=pc4[:], in1=lb2_sb[:])
            nc.sync.dma_start(out=out_d[:], in_=res[:])
    nc.compile()
    return nc


def _rsqrt(nc, pool, out_t, deg_t, tg):
    """out = rsqrt(max(deg, 1)) : ACT sqrt + DVE reciprocal + 1 Newton step."""
    shape = [deg_t.shape[0], deg_t.shape[1]]
    d1 = pool.tile(shape, F32, tag=f"rs_d{tg}")
    s = pool.tile(shape, F32, tag=f"rs_s{tg}")
    t = pool.tile(shape, F32, tag=f"rs_t{tg}")
    nc.vector.tensor_scalar_max(d1[:], deg_t[:], 1.0)
    nc.scalar.sqrt(s[:], d1[:])
    nc.vector.reciprocal(out_t[:], s[:])
    nc.vector.tensor_mul(out=t[:], in0=out_t[:], in1=out_t[:])
    nc.vector.tensor_mul(out=t[:], in0=t[:], in1=d1[:])
    nc.vector.tensor_scalar(out=t[:], in0=t[:], scalar1=-0.5, scalar2=1.5,
                            op0=ALU.mult, op1=ALU.add)
    nc.vector.tensor_mul(out=out_t[:], in0=out_t[:], in1=t[:])


def _bitonic_sort(nc, xs, xs2):
    """Ascending bitonic sort along the last axis (512) of [128, RBLK, 512].

    Direction-partitioned, all-forward access patterns: substage (k, d)
    compares (e, e+d); direction = bit k of e. Elements viewed as
    (x, t, i): e = x*2d + t*d + i; dir = bit (k-sub-1) of x.
    Ping-pongs xs/xs2; returns the buffer holding the result.
    """
    bufs = [xs, xs2]
    cur = 0
    W = 512

    def cmpex(src, dst, k, sub):
        d = 1 << sub
        # e = x*2d + t*d + i ; dir(e) = bit k of e = bit m of x
        m = k - sub - 1
        if k == 9:
            # all ascending
            vs = src[:].rearrange("p r (x two i) -> p r x two i", two=2, i=d)
            vd = dst[:].rearrange("p r (x two i) -> p r x two i", two=2, i=d)
            lo_s, hi_s = vs[:, :, :, 0], vs[:, :, :, 1]
            lo_d, hi_d = vd[:, :, :, 0], vd[:, :, :, 1]
            nc.vector.tensor_tensor(out=lo_d, in0=lo_s, in1=hi_s, op=ALU.min)
            nc.vector.tensor_tensor(out=hi_d, in0=lo_s, in1=hi_s, op=ALU.max)
            return
        zc = 1 << m
        vs = src[:].rearrange("p r (y db z two i) -> p r y db z two i",
                              db=2, z=zc, two=2, i=d)
        vd = dst[:].rearrange("p r (y db z two i) -> p r y db z two i",
                              db=2, z=zc, two=2, i=d)
        for db in range(2):
            lo_s, hi_s = vs[:, :, :, db, :, 0], vs[:, :, :, db, :, 1]
            lo_d, hi_d = vd[:, :, :, db, :, 0], vd[:, :, :, db, :, 1]
            op_lo = ALU.min if db == 0 else ALU.max
            op_hi = ALU.max if db == 0 else ALU.min
            nc.vector.tensor_tensor(out=lo_d, in0=lo_s, in1=hi_s, op=op_lo)
            nc.vector.tensor_tensor(out=hi_d, in0=lo_s, in1=hi_s, op=op_hi)

    for k in range(1, 10):
        for sub in range(k - 1, -1, -1):
            cmpex(bufs[cur], bufs[1 - cur], k, sub)
            cur = 1 - cur
    return bufs[cur]


# ======================= host side =======================

_NC_CACHE = {}


def _wrap(a, p):
    """Return [p, len(a)//p] with element i at [i % p, i // p]."""
    return np.ascontiguousarray(a.reshape(-1, p).T)


def _prep_edges(src, dst):
    """Sort/pad edges per (core, graph, bucket); returns (NB, s_list, d_list)
    where s_list[c] is the [128, GPC*PAIRS*2] bf16 array for core c."""
    src = np.asarray(src).astype(np.int64)
    dst = np.asarray(dst).astype(np.int64)
    E_G = NPER * DEG
    sl_all = (src % NPER).reshape(B, E_G)
    dl_all = (dst % NPER).reshape(B, E_G)
    sh = sl_all >= 128
    dh = dl_all >= 128
    bucket = dh.astype(np.int64) * 2 + sh.astype(np.int64)
    s_loc = sl_all % 128
    d_loc = dl_all % 128

    # bucket counts -> NB
    counts = np.zeros((B, 4), np.int64)
    for bk in range(4):
        counts[:, bk] = (bucket == bk).sum(axis=1)
    NB = int(np.ceil(counts.max() / 128))

    slots = 4 * NB * 128
    s_pad = np.full((B, slots), DUMMY, np.int16)
    d_pad = np.full((B, slots), DUMMY, np.int16)
    for g in range(B):
        order = np.argsort(bucket[g], kind="stable")
        sb = s_loc[g][order]
        db = d_loc[g][order]
        cnt = counts[g]
        off_in = 0
        for bk in range(4):
            n = cnt[bk]
            base = bk * NB * 128
            s_pad[g, base:base + n] = sb[off_in:off_in + n]
            d_pad[g, base:base + n] = db[off_in:off_in + n]
            off_in += n
    # edge slot i of block b: row i%128... block b = slot//128;
    # DVE layout [128, (gl, bp), 2]: value at [p, gl*PAIRS+bp, j] =
    # slot index (bp*2+j)*128 + p
    import ml_dtypes
    bf16 = ml_dtypes.bfloat16
    s_list, d_list = [], []
    nblk = 4 * NB
    for c in range(NCORES):
        sg = s_pad[c * GPC:(c + 1) * GPC].reshape(GPC, nblk, 128)
        dg = d_pad[c * GPC:(c + 1) * GPC].reshape(GPC, nblk, 128)
        # -> [128 p, gl, bp, j]
        s_list.append(np.ascontiguousarray(
            sg.reshape(GPC, nblk // 2, 2, 128).transpose(3, 0, 1, 2)
            .reshape(128, -1)).astype(bf16))
        d_list.append(np.ascontiguousarray(
            dg.reshape(GPC, nblk // 2, 2, 128).transpose(3, 0, 1, 2)
            .reshape(128, -1)).astype(bf16))
    return NB, s_list, d_list


def shard_inputs(inputs):
    import ml_dtypes
    bf16 = ml_dtypes.bfloat16
    z = np.asarray(inputs["z"]).astype(np.int64)
    NB, s_list, d_list = _prep_edges(inputs["src"], inputs["dst"])
    z_table = np.ascontiguousarray(np.asarray(inputs["z_table"], np.float32))
    biases = np.ascontiguousarray(np.asarray(inputs["biases"], np.float32))
    w1 = np.ascontiguousarray(np.asarray(inputs["conv1_w"], np.float32).reshape(16, 384))
    b1 = np.ascontiguousarray(
        np.asarray(inputs["conv1_b"], np.float32).reshape(16, 1))
    w2m = np.ascontiguousarray(
        np.asarray(inputs["conv2_w"], np.float32).transpose(0, 2, 1)
        .reshape(32, 80))
    b2 = np.ascontiguousarray(
        np.asarray(inputs["conv2_b"], np.float32).reshape(32, 1))
    lw1m = np.ascontiguousarray(
        np.asarray(inputs["lin1_w"], np.float32).reshape(128, 32, 11)
        .transpose(0, 2, 1).reshape(128, 352))
    lb1 = np.ascontiguousarray(
        np.asarray(inputs["lin1_b"], np.float32).reshape(128, 1))
    lw2 = np.ascontiguousarray(np.asarray(inputs["lin2_w"], np.float32).reshape(128, 1))
    lb2 = np.ascontiguousarray(
        np.asarray(inputs["lin2_b"], np.float32).reshape(1, 1))

    in_maps = []
    for c in range(NCORES):
        zl = z[c * NPC:(c + 1) * NPC]
        in_maps.append({
            "z_idx": np.tile(_wrap(zl.astype(np.int16), 16), (8, 1)),
            "s_arr": s_list[c], "d_arr": d_list[c],
            "z_table": z_table, "biases": biases,
            "w1": w1, "b1": b1, "w2m": w2m, "b2": b2,
            "lw1m": lw1m, "lb1": lb1, "lw2": lw2, "lb2": lb2,
        })
    return NB, in_maps


def kernel(**inputs):
    from concourse.bass_utils import run_bass_kernel_spmd
    NB, in_maps = shard_inputs(inputs)
    if NB not in _NC_CACHE:
        _NC_CACHE[NB] = build_nc(NB)
    nc = _NC_CACHE[NB]
    res = run_bass_kernel_spmd(nc, in_maps, core_ids=list(range(NCORES)))
    outs = [np.asarray(res.results[c]["out"], np.float32)
            for c in range(NCORES)]
    return np.concatenate(outs, axis=0)
